# revision 15
# baseline (speedup 1.0000x reference)
"""Trainium2 Bass kernel for nn_JaxGRU: encoder Dense -> GRU scan (T=100) -> output Dense.

Sharding: data-parallel, batch 256 -> 32 per core across 8 cores; weights replicated.

Per-core device program (all fp32):
  - encoder: h0 = relu(hist @ W_in + b_in) via PE, streaming W_in chunks from HBM
  - GRU scan, T=100 steps, everything SBUF-resident:
      * gh for the 3 gates via 3-way column-tiled matmuls (tile_position), with the
        a_t @ Wi + bi contribution fused into the r/z streams as a 9th contraction
        chunk, and bhn fused into the n stream via a ones-row chunk
      * sigmoid on ScalarE, r/z transposed to hidden-major via PE transposes,
        h_n+bhn transposed via VectorE 32x32 block transposes
      * i_n (+ bi_n) computed directly in hidden-major layout by a small matmul
      * gate arithmetic on VectorE at full 128-partition width
  - output Dense: outT = Wo.T @ outsT + bo
"""

import numpy as np

BS, HIST_LEN, FEAT = 256, 250, 32
T, D = 100, 32
HID, OUT = 1024, 64
NCORES = 8
B = BS // NCORES          # 32
G3 = 3 * HID              # 3072
HIST = HIST_LEN * FEAT    # 8000
KX = 63                   # encoder contraction chunks (8064 = 63*128, zero padded)
HISTP = KX * 128
KH = HID // 128           # 8
TB = T * B                # 3200
DA = D + 1                # 33: action dim augmented with a ones row

_PROGRAM = None


def _emit(tc, d):
    import concourse.bass as bass  # noqa: F401
    from concourse import mybir
    from concourse.bass import ts, ds
    from concourse.masks import make_identity

    AF = mybir.ActivationFunctionType
    f32 = mybir.dt.float32
    nc = tc.nc

    with (
        tc.tile_pool(name="const", bufs=1) as cpool,
        tc.tile_pool(name="dram", bufs=1, space="DRAM") as dpool,
    ):
        # ---- resident weights / inputs ----
        ATa_sb = cpool.tile([DA, TB], f32)
        nc.sync.dma_start(ATa_sb[:], d["ATa"][:])
        Wia_sb = cpool.tile([DA, G3], f32)
        nc.sync.dma_start(Wia_sb[:], d["Wia"][:])
        bhn_sb = cpool.tile([1, HID], f32)
        nc.sync.dma_start(bhn_sb[:], d["bhn"][:])
        bin_sb = cpool.tile([1, HID], f32)
        nc.sync.dma_start(bin_sb[:], d["b_in"][:])
        ident = cpool.tile([96, 96], f32)
        make_identity(nc, ident[:])
        ones_sb = cpool.tile([1, B], f32)
        nc.vector.memset(ones_sb[:], 1.0)
        histT_sb = cpool.tile([128, KX, B], f32)
        nc.sync.dma_start(histT_sb[:], d["histT"].rearrange("(g p) b -> p g b", p=128))
        Wh_sb = cpool.tile([128, KH, G3], f32)
        nc.sync.dma_start(Wh_sb[:], d["Wh"].rearrange("(g p) c -> p g c", p=128))

        outsT_d = dpool.tile([128, KH, TB], f32)

        with tc.tile_pool(name="hpool", bufs=2) as hpool:
            # ---- encoder: h0 = relu(hist @ W_in + b_in), output in hidden-major hT ----
            with (
                tc.tile_pool(name="winp", bufs=3) as winp,
                tc.tile_pool(name="encps", bufs=1, space="PSUM") as encps,
            ):
                ps_h = encps.tile([B, HID], f32)
                win_r = d["Win"].rearrange("(g p) h -> g p h", p=128)
                for g in range(KX):
                    wchunk = winp.tile([128, HID], f32, tag="win")
                    nc.sync.dma_start(wchunk[:], win_r[g])
                    for nh in range(2):
                        nc.tensor.matmul(
                            ps_h[:, ts(nh, 512)],
                            lhsT=histT_sb[:, g, :],
                            rhs=wchunk[:, ts(nh, 512)],
                            start=(g == 0),
                            stop=False,
                        )
                for nh in range(2):  # + b_in via ones row
                    nc.tensor.matmul(
                        ps_h[:, ts(nh, 512)],
                        lhsT=ones_sb[:],
                        rhs=bin_sb[:, ts(nh, 512)],
                        start=False,
                        stop=True,
                    )
                h0b = winp.tile([B, HID], f32, tag="h0b")
                nc.scalar.activation(h0b[:], ps_h[:], AF.Relu)
                ps_hT = encps.tile([128, KH, B], f32, tag="pshT")
                for g in range(KH):
                    nc.tensor.transpose(
                        ps_hT[:, g, :], h0b[:, ds(128 * g, 128)], ident[0:B, 0:B]
                    )
                hT = hpool.tile([128, KH, B], f32, tag="hT")
                nc.vector.tensor_copy(hT[:], ps_hT[:])

            # ---- GRU scan ----
            with (
                tc.tile_pool(name="gp", bufs=2) as gp,
                tc.tile_pool(name="scanps", bufs=2, space="PSUM") as sps,
                tc.tile_pool(name="tpps", bufs=1, space="PSUM") as tpps,
            ):
                for t in range(T):
                    at = ATa_sb[:, ts(t, B)]  # [33, 32] (last row = ones)
                    ps_g = sps.tile([96, HID], f32, tag="psg")
                    # gh = h @ Wh, 3 column groups (r, z, n) concurrent on PE
                    for k in range(KH):
                        for j in range(3):
                            for nh in range(2):
                                nc.tensor.matmul(
                                    ps_g[ds(32 * j, 32), ts(nh, 512)],
                                    lhsT=hT[:, k, :],
                                    rhs=Wh_sb[:, k, ds(1024 * j + 512 * nh, 512)],
                                    start=(k == 0),
                                    stop=False,
                                    tile_position=(0, 32 * j),
                                    skip_group_check=True,
                                )
                    # fuse a_t @ Wi + bi into r/z
                    for j in range(2):
                        for nh in range(2):
                            nc.tensor.matmul(
                                ps_g[ds(32 * j, 32), ts(nh, 512)],
                                lhsT=at,
                                rhs=Wia_sb[:, ds(1024 * j + 512 * nh, 512)],
                                start=False,
                                stop=True,
                                tile_position=(0, 32 * j),
                                skip_group_check=True,
                            )
                    # fuse + bhn into n via ones row
                    for nh in range(2):
                        nc.tensor.matmul(
                            ps_g[ds(64, 32), ts(nh, 512)],
                            lhsT=ones_sb[:],
                            rhs=bhn_sb[:, ts(nh, 512)],
                            start=False,
                            stop=True,
                            tile_position=(0, 64),
                            skip_group_check=True,
                        )
                    # i_nT (+ bi_n) directly in hidden-major layout
                    ps_i = sps.tile([128, KH, B], f32, tag="psi")
                    for g in range(KH):
                        nc.tensor.matmul(
                            ps_i[:, g, :],
                            lhsT=Wia_sb[:, ds(2 * HID + 128 * g, 128)],
                            rhs=at,
                            start=(g == 0),
                            stop=(g == KH - 1),
                        )
                    # r,z = sigmoid(gh_rz); hb = gh_n copied alongside
                    srzn = gp.tile([96, HID], f32, tag="srzn")
                    nc.scalar.activation(srzn[0:64, :], ps_g[0:64, :], AF.Sigmoid)
                    nc.scalar.copy(srzn[64:96, :], ps_g[64:96, :])
                    # transpose r,z,hb to hidden-major via PE: [96,128] -> [128,96]
                    # (chunk stride padded to 128 floats so no write crosses a
                    #  PSUM bank boundary)
                    ps_t = tpps.tile([128, KH, 128], f32, tag="pst")
                    for g in range(KH):
                        nc.tensor.transpose(
                            ps_t[:, g, 0:96],
                            srzn[:, ds(128 * g, 128)],
                            ident[0:96, 0:96],
                        )
                    rT = ps_t[:, :, 0:32]
                    zT = ps_t[:, :, 32:64]
                    # hb to SBUF (DVE cannot read two PSUM operands)
                    hbT = gp.tile([128, KH, B], f32, tag="hbT")
                    nc.scalar.copy(hbT[:], ps_t[:, :, 64:96])
                    # n = tanh(i_n + r * (h_n + bhn))
                    t1 = gp.tile([128, KH, B], f32, tag="t1")
                    nc.vector.tensor_mul(t1[:], rT, hbT[:])
                    t2 = gp.tile([128, KH, B], f32, tag="t2")
                    nc.vector.tensor_add(t2[:], t1[:], ps_i[:])
                    nT = gp.tile([128, KH, B], f32, tag="nT")
                    nc.scalar.activation(nT[:], t2[:], AF.Tanh)
                    # h' = z*h + (1-z)*n
                    zc = gp.tile([128, KH, B], f32, tag="zc")
                    nc.scalar.activation(zc[:], zT, AF.Copy, bias=1.0, scale=-1.0)
                    e1 = gp.tile([128, KH, B], f32, tag="e1")
                    nc.vector.tensor_mul(e1[:], zT, hT[:])
                    e2 = gp.tile([128, KH, B], f32, tag="e2")
                    nc.vector.tensor_mul(e2[:], zc[:], nT[:])
                    hT_new = hpool.tile([128, KH, B], f32, tag="hT")
                    nc.vector.tensor_add(hT_new[:], e1[:], e2[:])
                    nc.sync.dma_start(outsT_d[:, :, ts(t, B)], hT_new[:])
                    hT = hT_new

        # ---- output Dense: outT = Wo.T @ outsT + bo ----
        with (
            tc.tile_pool(name="op", bufs=2) as op,
            tc.tile_pool(name="outps", bufs=1, space="PSUM") as ops_,
        ):
            Wo_sb = op.tile([128, KH, OUT], f32, tag="wo")
            nc.sync.dma_start(Wo_sb[:], d["Wo"].rearrange("(g p) o -> p g o", p=128))
            bo_sb = op.tile([OUT, 1], f32, tag="bo")
            nc.sync.dma_start(bo_sb[:], d["bo"][:])
            ps_o = ops_.tile([OUT, TB], f32)
            for g in range(KH):
                oc = op.tile([128, TB], f32, tag="oc")
                nc.sync.dma_start(oc[:], outsT_d[:, g, :])
                for ns in range(7):
                    w = 512 if ns < 6 else TB - 6 * 512
                    nc.tensor.matmul(
                        ps_o[:, ds(512 * ns, w)],
                        lhsT=Wo_sb[:, g, :],
                        rhs=oc[:, ds(512 * ns, w)],
                        start=(g == 0),
                        stop=(g == KH - 1),
                    )
            out_sb = op.tile([OUT, TB], f32, tag="osb")
            nc.vector.tensor_scalar_add(out_sb[:], ps_o[:], bo_sb[:])
            nc.sync.dma_start(d["outT"][:], out_sb[:])


def build_program():
    """Build and bacc-compile the per-core Bass program (cached)."""
    global _PROGRAM
    if _PROGRAM is not None:
        return _PROGRAM
    import concourse.tile as tile
    from concourse import bacc, mybir

    f32 = mybir.dt.float32
    nc = bacc.Bacc("TRN2", target_bir_lowering=False, debug=False)
    d = {
        "histT": nc.dram_tensor("histT", [HISTP, B], f32, kind="ExternalInput").ap(),
        "ATa": nc.dram_tensor("ATa", [DA, TB], f32, kind="ExternalInput").ap(),
        "Win": nc.dram_tensor("Win", [HISTP, HID], f32, kind="ExternalInput").ap(),
        "Wh": nc.dram_tensor("Wh", [HID, G3], f32, kind="ExternalInput").ap(),
        "Wia": nc.dram_tensor("Wia", [DA, G3], f32, kind="ExternalInput").ap(),
        "bhn": nc.dram_tensor("bhn", [1, HID], f32, kind="ExternalInput").ap(),
        "b_in": nc.dram_tensor("b_in", [1, HID], f32, kind="ExternalInput").ap(),
        "Wo": nc.dram_tensor("Wo", [HID, OUT], f32, kind="ExternalInput").ap(),
        "bo": nc.dram_tensor("bo", [OUT, 1], f32, kind="ExternalInput").ap(),
        "outT": nc.dram_tensor("outT", [OUT, TB], f32, kind="ExternalOutput").ap(),
    }
    with tile.TileContext(nc) as tc:
        _emit(tc, d)
    nc.compile()
    _PROGRAM = nc
    return nc


def make_in_maps(inputs):
    """Host-side shard/layout prep: full inputs -> list of 8 per-core input dicts."""
    history = np.ascontiguousarray(np.asarray(inputs["history"], dtype=np.float32))
    action = np.ascontiguousarray(np.asarray(inputs["action"], dtype=np.float32))
    W_in = np.asarray(inputs["W_in"], dtype=np.float32)
    b_in = np.asarray(inputs["b_in"], dtype=np.float32)
    Wi = np.asarray(inputs["Wi"], dtype=np.float32)
    bi = np.asarray(inputs["bi"], dtype=np.float32)
    Wh = np.ascontiguousarray(np.asarray(inputs["Wh"], dtype=np.float32))
    bhn = np.asarray(inputs["bhn"], dtype=np.float32)
    Wo = np.ascontiguousarray(np.asarray(inputs["Wo"], dtype=np.float32))
    bo = np.asarray(inputs["bo"], dtype=np.float32)

    Win_p = np.zeros((HISTP, HID), np.float32)
    Win_p[:HIST] = W_in
    Wia = np.concatenate([Wi, bi[None, :]], axis=0)  # [33, 3072]
    Wia = np.ascontiguousarray(Wia)
    bhn_r = np.ascontiguousarray(bhn[None, :])
    b_in_r = np.ascontiguousarray(b_in[None, :])
    bo_r = np.ascontiguousarray(bo[:, None])

    in_maps = []
    for c in range(NCORES):
        sl = slice(c * B, (c + 1) * B)
        histT = np.zeros((HISTP, B), np.float32)
        histT[:HIST] = history[sl].reshape(B, HIST).T
        ATa = np.empty((DA, TB), np.float32)
        ATa[:D] = action[sl].transpose(2, 1, 0).reshape(D, TB)
        ATa[D] = 1.0
        in_maps.append(
            {
                "histT": histT,
                "ATa": np.ascontiguousarray(ATa),
                "Win": Win_p,
                "Wh": Wh,
                "Wia": Wia,
                "bhn": bhn_r,
                "b_in": b_in_r,
                "Wo": Wo,
                "bo": bo_r,
            }
        )
    return in_maps


def assemble_output(results):
    """Per-core outT [64, 3200] -> full [256, 100, 64] float32."""
    outs = []
    for c in range(NCORES):
        outT = results[c]["outT"]  # [OUT, TB]
        outs.append(outT.reshape(OUT, T, B).transpose(2, 1, 0))  # [B, T, OUT]
    return np.ascontiguousarray(np.concatenate(outs, axis=0).astype(np.float32))


def kernel(**inputs) -> np.ndarray:
    from concourse.bass_utils import run_bass_kernel_spmd

    nc = build_program()
    in_maps = make_in_maps(inputs)
    res = run_bass_kernel_spmd(nc, in_maps, core_ids=list(range(NCORES)))
    return assemble_output(res.results)
